# revision 38
# baseline (speedup 1.0000x reference)
"""MultiHeadAttention kernel for 8x TRN2 NeuronCores.

The reference module's einsum reduces the attention tensor over BOTH the
query and key axes (attn_mass = sum_{q,k} softmax(logits)_k), and softmax
rows sum to 1, so attn_mass == Lq exactly for every (batch, head). The
whole computation therefore collapses to

    out = (Lq * (V_heads @ Wv^T + bv)).reshape(N, L, E) @ Wo^T + bo

which is a single dense GEMM after folding the (block-diagonal) per-head
V-projection into the output projection:

    out = V_flat @ W_eff + b_eff
    W_eff[h*hd+a, n] = Lq * sum_b Wv[b, a] * Wo[n, h*hd+b]      (1024 x 1024)
    b_eff[n]         = Lq * sum_{h,b} Wo[n, h*hd+b] * bv[b] + bo[n]

The device kernel is the GEMM, row-sharded across 8 cores (512 rows per
core), computed in TRANSPOSED orientation: PSUM bank j holds output
columns j*128..(j+1)*128 on partitions x all 512 rows on the free dim,
accumulating lhsT = W-block j k-slabs against rhs = X^T k-slabs.

Everything rides bf16 (inputs, weights, output -- PSUM still accumulates
fp32; the 2e-2 tolerance leaves ~8x margin at the measured 2.6e-3 l2
error), halving HBM traffic vs fp32.  The schedule is built around the
bottlenecks measured in NTFF traces of earlier revisions:

  * HAM clock ramp: the PE runs at ~1.2 GHz until it has been busy
    ~4-5us, so a junk-matmul burst (memset data, no DMA deps) starts
    right after the preamble and is sized to drain just as the first
    real operands land (~11us); the PE then never idles long enough to
    re-cool the clock, and real matmuls stream at ~216-227ns each.
  * DMA: per-queue throughput collapses when all 8 cores run three
    HWDGE queues (24 chip-wide), so the bulk input streams on only TWO
    queues (sync + scalar), X slabs alternating with W half-chunks in
    exactly the order the matmul schedule consumes them; gpsimd's
    software-DGE queue is reserved for early-bank output DMAs.  MM_ORDER
    is availability-greedy against the measured per-piece arrival
    cadence (~0.8-1.3us per 128KB piece per queue).
  * Tail: output DMAs are spread over all three engines in bank
    completion order so they never jam behind input FIFO traffic.  The
    last bank evicts as ONE full-bank bias-add on vector -- the DVE is
    free-dim bound, so [128,512] costs the same ~742ns as any split --
    followed by a single full-row DMA on sync; routing half the
    eviction through the scalar engine was measured to add its ~0.8us
    semaphore wake lag to the critical path for zero ts gain.

Host side folds W_eff/b_eff in fp64, transposes V-shards to X^T, casts
to bf16, and upcasts the bf16 outputs back to fp32.
"""

import numpy as np
import ml_dtypes

import concourse.bass as bass
import concourse.bacc as bacc
import concourse.mybir as mybir
from concourse.tile import TileContext
from concourse.bass_utils import run_bass_kernel_spmd

N_CORES = 8
E = 1024            # embed dim == d_model
H, HD = 16, 64      # heads, head dim
ROWS = 4096         # N * L = 2 * 2048
RPC = ROWS // N_CORES   # rows per core = 512
P = 128             # SBUF partitions
KT = E // P         # 8 contraction slabs
JT = E // P         # 8 output-column banks

# Junk-matmul warm-up burst: keeps the PE busy (HAM ramp) from preamble
# exit until the first real operands land (~4.9us at the mid p-state,
# real matmuls start ~12.9us).  Sized generously: a >1us idle before
# the clock reaches full speed can pin the PE at half clock for
# several us, which costs far more than the extra junk.
N_JUNK_512 = 11
N_JUNK_128 = 2

# MM order matched to a strict two-queue input pipeline (sync/scalar
# alternate X slabs and W half-chunks, one piece landing every ~0.8us
# per queue; 16 chip-wide queues sustain much higher per-queue rates
# than 24).  Banks 0,1 lead, bank 2 follows, then the tail banks in
# arrival order; bank 6 is last.
MM_ORDER = [
    (0, 0), (0, 1), (1, 0), (1, 1), (0, 2), (1, 2), (0, 3), (1, 3),
    (2, 0), (2, 1), (2, 2), (2, 3), (7, 0), (7, 1), (7, 2), (7, 3),
    (0, 4), (1, 4), (0, 5), (1, 5), (7, 4), (7, 5), (7, 6), (0, 6),
    (1, 6), (3, 0), (3, 1), (3, 2), (3, 3), (2, 4), (2, 5), (2, 6),
    (2, 7), (0, 7), (1, 7), (7, 7), (3, 4), (3, 5), (3, 6), (3, 7),
] + [(j, k) for j in (4, 5, 6) for k in range(KT)]
# Bank completion order implied by MM_ORDER (evictions follow it).
EVICT_ORDER = [2, 0, 1, 7, 3, 4, 5, 6]

_NC_CACHE = {}
LAST_RESULTS = None  # BassKernelResults of the most recent device run


def _build():
    f32 = mybir.dt.float32
    bf16 = mybir.dt.bfloat16
    nc = bacc.Bacc(None, target_bir_lowering=False)
    xs = nc.declare_dram_parameter("xs", [E, RPC], bf16, isOutput=False)
    wc = nc.declare_dram_parameter("wc", [JT * P, E], bf16, isOutput=False)
    bw = nc.declare_dram_parameter("bw", [P, JT], f32, isOutput=False)
    outT = nc.declare_dram_parameter("outT", [E, RPC], bf16, isOutput=True)

    with TileContext(nc) as tc:
        with (
            tc.tile_pool(name="xp", bufs=1) as xp,
            tc.tile_pool(name="wp", bufs=1) as wp,
            tc.tile_pool(name="bp", bufs=1) as bp,
            tc.tile_pool(name="pp", bufs=1, space="PSUM") as pp,
            tc.tile_pool(name="op", bufs=1) as op,
        ):
            # Junk tile for the warm-up burst: memset needs no DMA and runs
            # first on gpsimd, so the PE can start right after the preamble
            # (a vector-side memset was measured to delay the burst ~1us).
            wm = bp.tile([P, RPC], bf16, name="wm", tag="wm")
            nc.gpsimd.memset(wm[:], 1.0)
            bias = bp.tile([P, JT], f32, name="bias", tag="bias")

            xts = [
                xp.tile([P, RPC], bf16, name=f"x{k}", tag=f"x{k}")
                for k in range(KT)
            ]

            def xslab(k):
                return xts[k][:, :]

            # W chunk tiles; wmap[(j, k)] = (tile, col offset).
            wmap = {}

            def wchunk(j, k0, k1, engine):
                t = wp.tile([P, (k1 - k0) * P], bf16, name=f"w{j}_{k0}{k1}",
                            tag=f"w{j}_{k0}{k1}")
                engine.dma_start(
                    out=t[:], in_=wc[j * P:(j + 1) * P, k0 * P:k1 * P]
                )
                for k in range(k0, k1):
                    wmap[(j, k)] = (t, (k - k0) * P)

            # --- DMA schedule ------------------------------------------
            # Strict two-queue input pipeline: the fp32 baseline showed
            # per-queue DMA rates collapse when all 8 cores run three
            # queues (24 chip-wide) but sustain ~2x more with two.  X
            # slabs and W half-chunks alternate so each bank's operands
            # land just ahead of its matmuls; gpsimd is reserved for
            # output DMAs only.
            wchunk(7, 0, 4, nc.gpsimd)
            wchunk(7, 4, 8, nc.gpsimd)
            nc.gpsimd.dma_start(out=bias[:], in_=bw[:, :])
            nc.sync.dma_start(out=xts[0][:], in_=xs[0:P, :])
            wchunk(0, 0, 4, nc.sync)
            nc.sync.dma_start(out=xts[2][:], in_=xs[2 * P:3 * P, :])
            wchunk(2, 0, 4, nc.sync)
            wchunk(0, 4, 8, nc.sync)
            nc.sync.dma_start(out=xts[4][:], in_=xs[4 * P:5 * P, :])
            nc.sync.dma_start(out=xts[6][:], in_=xs[6 * P:7 * P, :])
            wchunk(2, 4, 8, nc.sync)
            wchunk(4, 0, 4, nc.sync)
            wchunk(4, 4, 8, nc.sync)
            wchunk(6, 0, 4, nc.sync)
            wchunk(6, 4, 8, nc.sync)
            wchunk(1, 0, 4, nc.scalar)
            nc.scalar.dma_start(out=xts[1][:], in_=xs[P:2 * P, :])
            nc.scalar.dma_start(out=xts[3][:], in_=xs[3 * P:4 * P, :])
            wchunk(1, 4, 8, nc.scalar)
            nc.scalar.dma_start(out=xts[5][:], in_=xs[5 * P:6 * P, :])
            wchunk(3, 0, 4, nc.scalar)
            nc.scalar.dma_start(out=xts[7][:], in_=xs[7 * P:8 * P, :])
            wchunk(3, 4, 8, nc.scalar)
            wchunk(5, 0, 4, nc.scalar)
            wchunk(5, 4, 8, nc.scalar)

            ps = [
                pp.tile([P, RPC], f32, name=f"ps{j}", tag=f"ps{j}")
                for j in range(JT)
            ]

            # Warm-up burst: nonzero bf16 junk matmuls, no DMA deps.
            for i in range(N_JUNK_512):
                nc.tensor.matmul(
                    ps[i % JT], wm[:, 0:P], wm[:, :], start=True, stop=True
                )
            for i in range(N_JUNK_128):
                nc.tensor.matmul(
                    ps[(N_JUNK_512 + i) % JT][:, 0:P],
                    wm[:, 0:P], wm[:, 0:P], start=True, stop=True,
                )

            for j, k in MM_ORDER:
                t, off = wmap[(j, k)]
                nc.tensor.matmul(
                    ps[j],
                    t[:, off:off + P],
                    xslab(k),
                    start=(k == 0),
                    stop=(k == KT - 1),
                )

            # Evictions in bank-completion order: fused bias add
            # fp32->bf16 on vector, out DMAs routed to whichever queue is
            # drained when the bank completes (each engine's outs queue
            # naturally behind its remaining input FIFO traffic).  The
            # LAST bank (6) evicts split BY PARTITION (full 1KB DRAM
            # rows, unlike column halves whose 512B strided pieces
            # transfer ~2x slower) on vector + scalar-activation, with
            # out DMAs on sync + scalar in parallel.
            out_eng = {2: nc.gpsimd, 0: nc.gpsimd, 1: nc.gpsimd,
                       7: nc.gpsimd, 3: nc.sync, 4: nc.scalar,
                       5: nc.sync}
            for j in EVICT_ORDER[:-1]:
                o = op.tile([P, RPC], bf16, name=f"o{j}", tag=f"o{j}")
                nc.vector.tensor_scalar_add(o[:], ps[j], bias[:, j:j + 1])
                out_eng[j].dma_start(
                    out=outT[j * P:(j + 1) * P, :], in_=o[:]
                )
            # Bank 6 (last): ONE full-bank bias-add on vector -- the DVE
            # is free-dim bound, so [128,512] costs the same ~742ns as a
            # half -- then a single full-row DMA on sync.  (The earlier
            # vector+scalar split added the scalar engine's ~0.8us
            # semaphore wake lag to the critical path for zero ts gain.)
            o6 = op.tile([P, RPC], bf16, name="o6", tag="o6")
            nc.vector.tensor_scalar_add(o6[:], ps[6], bias[:, 6:7])
            nc.sync.dma_start(out=outT[6 * P:7 * P, :], in_=o6[:])
    nc.compile()
    return nc


def _get_nc():
    if "bf16" not in _NC_CACHE:
        _NC_CACHE["bf16"] = _build()
    return _NC_CACHE["bf16"]


def _prep_in_maps(V, Wv, bv, Wo, bo, lq):
    V = np.ascontiguousarray(np.asarray(V, dtype=np.float32))
    Wv64 = np.asarray(Wv, np.float64)
    Wo64 = np.asarray(Wo, np.float64)
    bv64 = np.asarray(bv, np.float64)
    bo64 = np.asarray(bo, np.float64)

    # Fold per-head V-projection + output projection + attention mass (== Lq).
    Wo_r = Wo64.reshape(E, H, HD)                       # [n, h, b]
    W_eff = lq * np.einsum("ba,nhb->han", Wv64, Wo_r, optimize=True)
    W_eff = W_eff.reshape(E, E).astype(np.float32)      # [k, n]
    b_eff = (lq * np.einsum("nhb,b->n", Wo_r, bv64) + bo64).astype(np.float32)

    # wc[j*P + p, k*P + c] = W_eff[k*P + p, j*P + c]  (lhsT blocks, natural)
    wc = np.ascontiguousarray(
        W_eff.reshape(KT, P, JT, P).transpose(2, 1, 0, 3).reshape(JT * P, E)
    ).astype(ml_dtypes.bfloat16)
    bw_blk = np.ascontiguousarray(b_eff.reshape(JT, P).T)   # [p, j] fp32

    X = V.reshape(ROWS, E)
    in_maps = []
    for i in range(N_CORES):
        xs_i = np.ascontiguousarray(
            X[i * RPC:(i + 1) * RPC, :].T.astype(ml_dtypes.bfloat16)
        )
        in_maps.append({"xs": xs_i, "wc": wc, "bw": bw_blk})
    return in_maps


def kernel(Q, K, V, Wq, bq, Wk, bk, Wv, bv, Wo, bo, **_unused):
    global LAST_RESULTS
    n, L, e = np.asarray(V).shape
    lq = float(np.asarray(Q).shape[1])
    in_maps = _prep_in_maps(V, Wv, bv, Wo, bo, lq)
    nc = _get_nc()
    LAST_RESULTS = run_bass_kernel_spmd(nc, in_maps, list(range(N_CORES)))
    out = np.concatenate(
        [
            LAST_RESULTS.results[i]["outT"].astype(np.float32).T
            for i in range(N_CORES)
        ],
        axis=0,
    )
    return np.ascontiguousarray(out).reshape(n, L, E)


# revision 39
# speedup vs baseline: 1.0343x; 1.0343x over previous
"""MultiHeadAttention kernel for 8x TRN2 NeuronCores.

The reference module's einsum reduces the attention tensor over BOTH the
query and key axes (attn_mass = sum_{q,k} softmax(logits)_k), and softmax
rows sum to 1, so attn_mass == Lq exactly for every (batch, head). The
whole computation therefore collapses to

    out = (Lq * (V_heads @ Wv^T + bv)).reshape(N, L, E) @ Wo^T + bo

which is a single dense GEMM after folding the (block-diagonal) per-head
V-projection into the output projection:

    out = V_flat @ W_eff + b_eff
    W_eff[h*hd+a, n] = Lq * sum_b Wv[b, a] * Wo[n, h*hd+b]      (1024 x 1024)
    b_eff[n]         = Lq * sum_{h,b} Wo[n, h*hd+b] * bv[b] + bo[n]

The device kernel is the GEMM, row-sharded across 8 cores (512 rows per
core), computed in TRANSPOSED orientation: PSUM bank j holds output
columns j*128..(j+1)*128 on partitions x all 512 rows on the free dim,
accumulating lhsT = W-block j k-slabs against rhs = X^T k-slabs.

Everything rides bf16 (inputs, weights, output -- PSUM still accumulates
fp32; the 2e-2 tolerance leaves ~8x margin at the measured 2.6e-3 l2
error), halving HBM traffic vs fp32.  The schedule is built around the
bottlenecks measured in NTFF traces of earlier revisions:

  * HAM clock ramp: the PE runs at ~1.2 GHz until it has been busy
    ~4-5us, so a junk-matmul burst (memset data, no DMA deps) starts
    right after the preamble and is sized to drain just as the first
    real operands land (~11us); the PE then never idles long enough to
    re-cool the clock, and real matmuls stream at ~216-227ns each.
  * DMA: per-queue throughput collapses when all 8 cores run three
    busy queues (24 chip-wide), so the bulk input streams on TWO HWDGE
    queues (sync + scalar), X slabs alternating with W half-chunks in
    exactly the order the matmul schedule consumes them; gpsimd's
    software-DGE queue carries only W7 + bias early (so bank 7 can fill
    the chip-wide congestion dip at ~15.5-18us where both HWDGE queues
    stall simultaneously) and the early-bank output DMAs late.
    MM_ORDER is availability-greedy against the measured per-piece
    arrival cadence (~0.8-1.3us per 128KB piece per queue), with the X
    tails consumed as late as possible since every bank needs them.
  * Tail: output DMAs are spread over all three engines in bank
    completion order so they never jam behind input FIFO traffic.  The
    last bank evicts as ONE full-bank bias-add on vector -- the DVE is
    free-dim bound, so [128,512] costs the same ~742ns as any split --
    followed by a single full-row DMA on sync; routing half the
    eviction through the scalar engine was measured to add its ~0.8us
    semaphore wake lag to the critical path for zero ts gain.

Host side folds W_eff/b_eff in fp64, transposes V-shards to X^T, casts
to bf16, and upcasts the bf16 outputs back to fp32.
"""

import numpy as np
import ml_dtypes

import concourse.bass as bass
import concourse.bacc as bacc
import concourse.mybir as mybir
from concourse.tile import TileContext
from concourse.bass_utils import run_bass_kernel_spmd

N_CORES = 8
E = 1024            # embed dim == d_model
H, HD = 16, 64      # heads, head dim
ROWS = 4096         # N * L = 2 * 2048
RPC = ROWS // N_CORES   # rows per core = 512
P = 128             # SBUF partitions
KT = E // P         # 8 contraction slabs
JT = E // P         # 8 output-column banks

# Junk-matmul warm-up burst: keeps the PE busy (HAM ramp) from preamble
# exit until the first real operands land (~4.9us at the mid p-state,
# real matmuls start ~12.9us).  Sized generously: a >1us idle before
# the clock reaches full speed can pin the PE at half clock for
# several us, which costs far more than the extra junk.
N_JUNK_512 = 11
N_JUNK_128 = 2

# MM order matched to a strict two-queue input pipeline (sync/scalar
# alternate X slabs and W half-chunks, one piece landing every ~0.8us
# per queue; 16 chip-wide queues sustain much higher per-queue rates
# than 24).  Banks 0,1 lead, bank 2 follows, then the tail banks in
# arrival order; bank 6 is last.
MM_ORDER = [
    (0, 0), (0, 1), (1, 0), (1, 1), (0, 2), (1, 2), (0, 3), (1, 3),
    (2, 0), (2, 1), (2, 2), (2, 3), (7, 0), (7, 1), (7, 2), (7, 3),
    (0, 4), (1, 4), (0, 5), (1, 5), (7, 4), (7, 5), (7, 6), (0, 6),
    (1, 6), (3, 0), (3, 1), (3, 2), (3, 3), (2, 4), (2, 5), (2, 6),
    (2, 7), (0, 7), (1, 7), (7, 7), (3, 4), (3, 5), (3, 6), (3, 7),
] + [(j, k) for j in (4, 5, 6) for k in range(KT)]
# Bank completion order implied by MM_ORDER (evictions follow it).
EVICT_ORDER = [2, 0, 1, 7, 3, 4, 5, 6]

_NC_CACHE = {}
LAST_RESULTS = None  # BassKernelResults of the most recent device run


def _build():
    f32 = mybir.dt.float32
    bf16 = mybir.dt.bfloat16
    nc = bacc.Bacc(None, target_bir_lowering=False)
    xs = nc.declare_dram_parameter("xs", [E, RPC], bf16, isOutput=False)
    wc = nc.declare_dram_parameter("wc", [JT * P, E], bf16, isOutput=False)
    bw = nc.declare_dram_parameter("bw", [P, JT], f32, isOutput=False)
    outT = nc.declare_dram_parameter("outT", [E, RPC], bf16, isOutput=True)

    with TileContext(nc) as tc:
        with (
            tc.tile_pool(name="xp", bufs=1) as xp,
            tc.tile_pool(name="wp", bufs=1) as wp,
            tc.tile_pool(name="bp", bufs=1) as bp,
            tc.tile_pool(name="pp", bufs=1, space="PSUM") as pp,
            tc.tile_pool(name="op", bufs=1) as op,
        ):
            # Junk tile for the warm-up burst: memset needs no DMA and runs
            # first on gpsimd, so the PE can start right after the preamble
            # (a vector-side memset was measured to delay the burst ~1us).
            wm = bp.tile([P, RPC], bf16, name="wm", tag="wm")
            nc.gpsimd.memset(wm[:], 1.0)
            bias = bp.tile([P, JT], f32, name="bias", tag="bias")

            xts = [
                xp.tile([P, RPC], bf16, name=f"x{k}", tag=f"x{k}")
                for k in range(KT)
            ]

            def xslab(k):
                return xts[k][:, :]

            # W chunk tiles; wmap[(j, k)] = (tile, col offset).
            wmap = {}

            def wchunk(j, k0, k1, engine):
                t = wp.tile([P, (k1 - k0) * P], bf16, name=f"w{j}_{k0}{k1}",
                            tag=f"w{j}_{k0}{k1}")
                engine.dma_start(
                    out=t[:], in_=wc[j * P:(j + 1) * P, k0 * P:k1 * P]
                )
                for k in range(k0, k1):
                    wmap[(j, k)] = (t, (k - k0) * P)

            # --- DMA schedule ------------------------------------------
            # Strict two-queue input pipeline: the fp32 baseline showed
            # per-queue DMA rates collapse when all 8 cores run three
            # queues (24 chip-wide) but sustain ~2x more with two.  X
            # slabs and W half-chunks alternate so each bank's operands
            # land just ahead of its matmuls; gpsimd is reserved for
            # output DMAs only.
            wchunk(7, 0, 4, nc.gpsimd)
            wchunk(7, 4, 8, nc.gpsimd)
            nc.gpsimd.dma_start(out=bias[:], in_=bw[:, :])
            nc.sync.dma_start(out=xts[0][:], in_=xs[0:P, :])
            wchunk(0, 0, 4, nc.sync)
            nc.sync.dma_start(out=xts[2][:], in_=xs[2 * P:3 * P, :])
            wchunk(2, 0, 4, nc.sync)
            wchunk(0, 4, 8, nc.sync)
            nc.sync.dma_start(out=xts[4][:], in_=xs[4 * P:5 * P, :])
            nc.sync.dma_start(out=xts[6][:], in_=xs[6 * P:7 * P, :])
            wchunk(2, 4, 8, nc.sync)
            wchunk(4, 0, 4, nc.sync)
            wchunk(4, 4, 8, nc.sync)
            wchunk(6, 0, 4, nc.sync)
            wchunk(6, 4, 8, nc.sync)
            wchunk(1, 0, 4, nc.scalar)
            nc.scalar.dma_start(out=xts[1][:], in_=xs[P:2 * P, :])
            nc.scalar.dma_start(out=xts[3][:], in_=xs[3 * P:4 * P, :])
            wchunk(1, 4, 8, nc.scalar)
            nc.scalar.dma_start(out=xts[5][:], in_=xs[5 * P:6 * P, :])
            wchunk(3, 0, 4, nc.scalar)
            nc.scalar.dma_start(out=xts[7][:], in_=xs[7 * P:8 * P, :])
            wchunk(3, 4, 8, nc.scalar)
            wchunk(5, 0, 4, nc.scalar)
            wchunk(5, 4, 8, nc.scalar)

            ps = [
                pp.tile([P, RPC], f32, name=f"ps{j}", tag=f"ps{j}")
                for j in range(JT)
            ]

            # Warm-up burst: nonzero bf16 junk matmuls, no DMA deps.
            for i in range(N_JUNK_512):
                nc.tensor.matmul(
                    ps[i % JT], wm[:, 0:P], wm[:, :], start=True, stop=True
                )
            for i in range(N_JUNK_128):
                nc.tensor.matmul(
                    ps[(N_JUNK_512 + i) % JT][:, 0:P],
                    wm[:, 0:P], wm[:, 0:P], start=True, stop=True,
                )

            for j, k in MM_ORDER:
                t, off = wmap[(j, k)]
                nc.tensor.matmul(
                    ps[j],
                    t[:, off:off + P],
                    xslab(k),
                    start=(k == 0),
                    stop=(k == KT - 1),
                )

            # Evictions in bank-completion order: fused bias add
            # fp32->bf16 on vector, out DMAs routed to whichever queue is
            # drained when the bank completes (each engine's outs queue
            # naturally behind its remaining input FIFO traffic).  The
            # LAST bank (6) evicts split BY PARTITION (full 1KB DRAM
            # rows, unlike column halves whose 512B strided pieces
            # transfer ~2x slower) on vector + scalar-activation, with
            # out DMAs on sync + scalar in parallel.
            out_eng = {2: nc.gpsimd, 0: nc.gpsimd, 1: nc.gpsimd,
                       7: nc.gpsimd, 3: nc.sync, 4: nc.scalar,
                       5: nc.sync}
            for j in EVICT_ORDER[:-1]:
                o = op.tile([P, RPC], bf16, name=f"o{j}", tag=f"o{j}")
                nc.vector.tensor_scalar_add(o[:], ps[j], bias[:, j:j + 1])
                out_eng[j].dma_start(
                    out=outT[j * P:(j + 1) * P, :], in_=o[:]
                )
            # Bank 6 (last): ONE full-bank bias-add on vector -- the DVE
            # is free-dim bound, so [128,512] costs the same ~742ns as a
            # half -- then a single full-row DMA on sync.  (The earlier
            # vector+scalar split added the scalar engine's ~0.8us
            # semaphore wake lag to the critical path for zero ts gain.)
            o6 = op.tile([P, RPC], bf16, name="o6", tag="o6")
            nc.vector.tensor_scalar_add(o6[:], ps[6], bias[:, 6:7])
            nc.sync.dma_start(out=outT[6 * P:7 * P, :], in_=o6[:])
    nc.compile()
    return nc


def _get_nc():
    if "bf16" not in _NC_CACHE:
        _NC_CACHE["bf16"] = _build()
    return _NC_CACHE["bf16"]


def _prep_in_maps(V, Wv, bv, Wo, bo, lq):
    V = np.ascontiguousarray(np.asarray(V, dtype=np.float32))
    Wv64 = np.asarray(Wv, np.float64)
    Wo64 = np.asarray(Wo, np.float64)
    bv64 = np.asarray(bv, np.float64)
    bo64 = np.asarray(bo, np.float64)

    # Fold per-head V-projection + output projection + attention mass (== Lq).
    Wo_r = Wo64.reshape(E, H, HD)                       # [n, h, b]
    W_eff = lq * np.einsum("ba,nhb->han", Wv64, Wo_r, optimize=True)
    W_eff = W_eff.reshape(E, E).astype(np.float32)      # [k, n]
    b_eff = (lq * np.einsum("nhb,b->n", Wo_r, bv64) + bo64).astype(np.float32)

    # wc[j*P + p, k*P + c] = W_eff[k*P + p, j*P + c]  (lhsT blocks, natural)
    wc = np.ascontiguousarray(
        W_eff.reshape(KT, P, JT, P).transpose(2, 1, 0, 3).reshape(JT * P, E)
    ).astype(ml_dtypes.bfloat16)
    bw_blk = np.ascontiguousarray(b_eff.reshape(JT, P).T)   # [p, j] fp32

    X = V.reshape(ROWS, E)
    in_maps = []
    for i in range(N_CORES):
        xs_i = np.ascontiguousarray(
            X[i * RPC:(i + 1) * RPC, :].T.astype(ml_dtypes.bfloat16)
        )
        in_maps.append({"xs": xs_i, "wc": wc, "bw": bw_blk})
    return in_maps


def kernel(Q, K, V, Wq, bq, Wk, bk, Wv, bv, Wo, bo, **_unused):
    global LAST_RESULTS
    n, L, e = np.asarray(V).shape
    lq = float(np.asarray(Q).shape[1])
    in_maps = _prep_in_maps(V, Wv, bv, Wo, bo, lq)
    nc = _get_nc()
    LAST_RESULTS = run_bass_kernel_spmd(nc, in_maps, list(range(N_CORES)))
    out = np.concatenate(
        [
            LAST_RESULTS.results[i]["outT"].astype(np.float32).T
            for i in range(N_CORES)
        ],
        axis=0,
    )
    return np.ascontiguousarray(out).reshape(n, L, E)
